# revision 1
# baseline (speedup 1.0000x reference)
"""Trainium2 Bass kernel for nn_DirectionalWedgeBias.

Computes, per (batch b, head h):
    v      = x[b].reshape(T, H, Dh)[:, h, :]          # [T, Dh]
    v_hat  = v / max(||v||_2, eps)  (row-wise)
    S      = A[h] - A[h]^T                            # [Dh, Dh]
    wedge  = (v_hat @ S) @ v_hat^T                    # [T, T]

Full shapes: x [2, 2048, 1024] f32, A [16, 64, 64] f32 -> out [2, 16, 2048, 2048] f32.

Sharding: 32 independent (b, h) pairs split 4-per-core across 8 NeuronCores
(data + head parallel; the tiny skew-symmetric S is replicated/sliced with the
heads). Host pre-slices x into per-core [4, T, Dh] blocks, forms S = A - A^T,
and re-stacks the per-core [4, T, T] results.

Per-core dataflow (Tile framework):
  - load v [2048, 64] as [128 parts, 16, 64]; row-normalize on DVE
    (square+reduce, ACT sqrt, DVE reciprocal, one broadcast multiply)
  - PE-transpose to vT [64, 2048] (Dh on partitions), f32r-rounded
  - SvT [64, 2048] = matmul(lhsT=S, rhs=vT), float32r (1 cyc/row vs 4 for
    fp32; measured rel err ~2e-4 against the fp32 reference)
  - wedge m-tiles: 4 matmuls (N=512, K=64) per [128, 2048] row block; PSUM
    evacuation alternates ScalarE/VectorE; two m-tiles share one staging tile
    so stores are 2 MiB each, alternating between the HWDGE (sync) ring and
    SWDGE (gpsimd) to overlap issue overheads and halve Q7 descriptor work
  - wedge PSUM pool is 3 slots x [128,1024] (6 banks) + 2 x [64,512] for
    transposes/Sv, so the PE runs up to 3 half-tiles ahead of the copies
  - pipeline fill: pair 0 spreads its x-chunk loads over the three DMA issue
    paths and stores its first half-tiles individually
  - walrus encodes at most ONE semaphore wait on most instructions (and two
    on EventSemaphore), so `_spill_waits` post-processes the Tile-scheduled
    BIR, hoisting excess waits onto preceding same-engine EventSemaphores
    (sequencers run in order, so this is semantics-preserving)

Cost-model (CoreSim) per-core time: ~121.7 us (engine busy: DVE/SP/Pool
~102-103 us each -- byte-bound in the model); the shared-HBM write floor for
the 64 MiB/core output is ~187 us at ~358 GB/s per core, so real silicon
likely lands at 150-190 us, write-bandwidth-bound.
"""

import numpy as np

B = 2
T = 2048
D = 1024
H = 16
Dh = 64
N_CORES = 8
PAIRS = (B * H) // N_CORES  # 4 per core
P = 128  # SBUF partitions

_COMPILED = {}

# test-harness knobs (default off; harness calls kernel() with these untouched)
TRACE = False
MM_DTYPE = "float32r"
LAST_RESULT = None


def _build_nc(pairs=PAIRS, t=T, mm_dtype_name="float32r", spill=True, repeat=1):
    _import_concourse()
    from contextlib import ExitStack

    import concourse.bass as bass
    import concourse.tile as tile
    from concourse import mybir

    f32 = mybir.dt.float32
    mmdt = getattr(mybir.dt, mm_dtype_name)
    nt = t // P  # t-tiles per pair
    ng = t // 512  # 512-wide col groups

    def mm_ap(ap):
        return ap.bitcast(mmdt) if mmdt is not f32 else ap

    nc = bass.Bass()
    x_in = nc.declare_dram_parameter("x", [pairs, t, Dh], f32, isOutput=False)
    s_in = nc.declare_dram_parameter("s", [pairs, Dh, Dh], f32, isOutput=False)
    id_in = nc.declare_dram_parameter("ident", [P, P], f32, isOutput=False)
    out_d = nc.declare_dram_parameter("out", [pairs, t, t], f32, isOutput=True)

    with ExitStack() as ctx:
        tc = ctx.enter_context(tile.TileContext(nc))
        const_pool = ctx.enter_context(tc.tile_pool(name="const", bufs=1))
        stage_pool = ctx.enter_context(tc.tile_pool(name="stage", bufs=2))
        pair_pool = ctx.enter_context(tc.tile_pool(name="pair", bufs=2))
        norm_pool = ctx.enter_context(tc.tile_pool(name="norm", bufs=2))
        psw_pool = ctx.enter_context(tc.tile_pool(name="psw", bufs=3, space="PSUM"))
        pst_pool = ctx.enter_context(tc.tile_pool(name="pst", bufs=2, space="PSUM"))
        out_pool = ctx.enter_context(tc.tile_pool(name="outb", bufs=6))

        # identity: DMA-landed, staged through ACT so matmuls only wait on ACT
        id_dma = const_pool.tile([P, P], f32)
        nc.sync.dma_start(out=id_dma, in_=id_in[:, :])
        identity = const_pool.tile([P, P], f32)
        nc.scalar.copy(identity, id_dma)
        # warmup matmul: absorbs the ACT(identity) wait so the first real
        # transpose only needs its DVE wait
        ps_warm = pst_pool.tile([Dh, 512], f32, tag="pst")
        nc.tensor.matmul(
            ps_warm[:1, :1],
            lhsT=identity[:1, :1],
            rhs=identity[:1, :1],
            start=True,
            stop=True,
        )

        for p in [q for _ in range(repeat) for q in range(pairs)]:
            # ---- S (precomputed skew-symmetric), staged through ACT ----
            s_dma = stage_pool.tile([Dh, Dh], f32, tag="sdma")
            nc.scalar.dma_start(out=s_dma, in_=s_in[p])
            s_sb = pair_pool.tile([Dh, Dh], f32, tag="s")
            nc.scalar.copy(mm_ap(s_sb[:]), s_dma)

            # ---- load v as [128, nt, 64], chunked per 512-row group so the
            #      square/reduce work overlaps the remaining loads ----
            v_sb = pair_pool.tile([P, nt, Dh], f32, tag="v")
            vsq = norm_pool.tile([P, nt, Dh], f32, tag="vsq")
            sumsq = norm_pool.tile([P, nt], f32, tag="ss")
            gn = nt // ng  # n-tiles per group (4)
            for g in range(ng):
                # pair 0 is the pipeline fill: spread its chunk loads over
                # the three idle DMA issue paths so they land concurrently
                if p == 0:
                    ld = (nc.sync, nc.gpsimd, nc.scalar, nc.gpsimd)[g % 4]
                else:
                    ld = nc.scalar
                ld.dma_start(
                    out=v_sb[:, g * gn : (g + 1) * gn, :],
                    in_=x_in[p][g * 512 : (g + 1) * 512, :].rearrange(
                        "(n p) d -> p n d", p=P
                    ),
                )
                nc.vector.tensor_mul(
                    vsq[:, g * gn : (g + 1) * gn, :],
                    v_sb[:, g * gn : (g + 1) * gn, :],
                    v_sb[:, g * gn : (g + 1) * gn, :],
                )
                nc.vector.reduce_sum(
                    sumsq[:, g * gn : (g + 1) * gn],
                    vsq[:, g * gn : (g + 1) * gn, :],
                    axis=mybir.AxisListType.X,
                )
            nrm = norm_pool.tile([P, nt], f32, tag="nrm")
            nc.scalar.activation(nrm, sumsq, mybir.ActivationFunctionType.Sqrt)
            rinv = norm_pool.tile([P, nt], f32, tag="rinv")
            nc.vector.reciprocal(rinv, nrm)

            # per group: normalize (fresh DVE-owned tile), PE-transpose,
            # evacuate, and immediately form that group's SvT slice so the
            # first wedge tiles can start before later groups finish
            v_hat = pair_pool.tile([P, nt, Dh], f32, tag="vhat")
            vt_sb = pair_pool.tile([Dh, t], f32, tag="vt")
            svt_sb = pair_pool.tile([Dh, t], f32, tag="svt")
            for g in range(ng):
                rb = (
                    rinv[:, g * gn : (g + 1) * gn]
                    .unsqueeze(-1)
                    .broadcast_to((P, gn, Dh))
                )
                nc.vector.tensor_mul(
                    v_hat[:, g * gn : (g + 1) * gn, :],
                    v_sb[:, g * gn : (g + 1) * gn, :],
                    rb,
                )
                ps_vt = pst_pool.tile([Dh, 512], f32, tag="pst")
                for j in range(gn):
                    n = g * gn + j
                    nc.tensor.transpose(
                        ps_vt[:, j * P : (j + 1) * P], v_hat[:, n, :], identity
                    )
                nc.vector.tensor_copy(mm_ap(vt_sb[:, g * 512 : (g + 1) * 512]), ps_vt)
                ps_sv = pst_pool.tile([Dh, 512], f32, tag="pst")
                nc.tensor.matmul(
                    ps_sv,
                    lhsT=mm_ap(s_sb[:]),
                    rhs=mm_ap(vt_sb[:, g * 512 : (g + 1) * 512]),
                    start=True,
                    stop=True,
                )
                nc.scalar.copy(mm_ap(svt_sb[:, g * 512 : (g + 1) * 512]), ps_sv)

            # ---- wedge tiles: [128, W] halves into a [128, 2W] out tile;
            #      evacuation alternates ACT/DVE; 1 MiB stores alternate
            #      between the HWDGE (sync) ring and SWDGE (gpsimd) ----
            W = 1024 if ng % 2 == 0 else 512
            wq = W // 512
            halves = t // W
            first_pair = p == 0 and repeat == 1
            for mm in range(0, nt, 2):
                # two m-tiles share one staging tile -> one 2 MiB store
                ob = out_pool.tile([P, 2, t], f32, tag="ob")
                fill = first_pair and mm < 8
                for ms in range(2):
                    m = mm + ms
                    for h in range(halves):
                        ps_w = psw_pool.tile([P, W], f32, tag="psw")
                        for q in range(wq):
                            g = h * wq + q
                            nc.tensor.matmul(
                                ps_w[:, q * 512 : (q + 1) * 512],
                                lhsT=mm_ap(svt_sb[:, m * P : (m + 1) * P]),
                                rhs=mm_ap(vt_sb[:, g * 512 : (g + 1) * 512]),
                                start=True,
                                stop=True,
                            )
                        dst = ob[:, ms, h * W : (h + 1) * W]
                        if (h + ms) % 2 == 0:
                            nc.scalar.copy(dst, ps_w)
                        else:
                            nc.vector.tensor_copy(dst, ps_w)
                        if fill:
                            # pipeline fill: store each half as soon as copied
                            eng = nc.sync if (m + h) % 2 == 0 else nc.gpsimd
                            eng.dma_start(
                                out=out_d[
                                    p, m * P : (m + 1) * P, h * W : (h + 1) * W
                                ],
                                in_=dst,
                            )
                if not fill:
                    last_group = p == pairs - 1 and mm == nt - 2
                    if last_group:
                        # pipeline drain: split the final store across both
                        # rings so the kernel-tail barrier waits half as long
                        for ms2, eng in ((0, nc.gpsimd), (1, nc.sync)):
                            eng.dma_start(
                                out=out_d[p, (mm + ms2) * P : (mm + ms2 + 1) * P, :],
                                in_=ob[:, ms2, :],
                            )
                    else:
                        eng = nc.sync if (mm // 2) % 2 == 0 else nc.gpsimd
                        eng.dma_start(
                            out=out_d[p][mm * P : (mm + 2) * P, :].rearrange(
                                "(m2 r) c -> r m2 c", m2=2
                            ),
                            in_=ob,
                        )

    if spill:
        _spill_waits(nc)
    return nc


def _spill_waits(nc, multi_ok=("EventSemaphore",), max_keep=1):
    """Walrus encodes at most one sync-wait on Matmult (embedded weight load)
    and DMACopy; move extra waits onto a preceding same-engine EventSemaphore
    (which supports many waits). The engine sequencer processes instructions
    in order, so a preceding wait is semantically identical."""
    from concourse import mybir

    n_spilled = 0
    for f in nc.m.functions:
        for bb in f.blocks:
            il = bb.instructions
            out = []
            for inst in il:
                si = getattr(inst, "sync_info", None)
                waits = list((si.on_wait if si else None) or [])
                cap = 2 if inst.opcode in multi_ok else max_keep
                if len(waits) > cap:
                    moved, keep = waits[:-max_keep], waits[-max_keep:]
                    for k in range(0, len(moved), 2):
                        es = mybir.InstEventSemaphore(
                            name=f"{inst.name}-wspill{k}",
                            engine=inst.engine,
                            ins=[],
                            outs=[],
                            sync_info=mybir.SyncInfo(
                                on_wait=moved[k : k + 2], on_update=[]
                            ),
                        )
                        out.append(es)
                    inst.sync_info = mybir.SyncInfo(
                        on_wait=keep, on_update=list(si.on_update or [])
                    )
                    n_spilled += 1
                out.append(inst)
            il[:] = out
    return n_spilled


def _import_concourse():
    try:
        import concourse  # noqa: F401
    except ImportError:
        import sys

        for p in ("/opt/trn_rl_repo", "/root/.axon_site/_ro/trn_rl_repo"):
            if p not in sys.path:
                sys.path.insert(0, p)


def _ensure_device_backend():
    """If the process pinned JAX_PLATFORMS to cpu, lift the pin so the
    NeuronCores (axon platform) are reachable for the kernel run."""
    import os

    plats = os.environ.get("JAX_PLATFORMS", "")
    if plats and "axon" not in plats and "neuron" not in plats:
        os.environ["JAX_PLATFORMS"] = ""
        try:
            import jax

            jax.extend.backend.clear_backends()
        except Exception:
            pass


def kernel(x, A, window_size=None):
    _import_concourse()
    _ensure_device_backend()
    from concourse.bass_utils import run_bass_kernel_spmd

    x = np.ascontiguousarray(x, dtype=np.float32)
    A = np.ascontiguousarray(A, dtype=np.float32)
    assert x.shape == (B, T, D) and A.shape == (H, Dh, Dh)

    nc = _COMPILED.get(MM_DTYPE)
    if nc is None:
        nc = _build_nc(mm_dtype_name=MM_DTYPE)
        _COMPILED[MM_DTYPE] = nc

    # x[b, t, h*64:(h+1)*64] per (b,h) pair; pair index bh = b*H + h.
    xv = x.reshape(B, T, H, Dh).transpose(0, 2, 1, 3).reshape(B * H, T, Dh)
    S = (A - np.swapaxes(A, -1, -2)).astype(np.float32)  # replicated with heads
    S_all = np.tile(S, (B, 1, 1))
    ident = np.eye(P, dtype=np.float32)
    in_maps = []
    for c in range(N_CORES):
        sl = slice(c * PAIRS, (c + 1) * PAIRS)
        in_maps.append(
            {
                "x": np.ascontiguousarray(xv[sl]),
                "s": np.ascontiguousarray(S_all[sl]),
                "ident": ident,
            }
        )
    res = run_bass_kernel_spmd(nc, in_maps, list(range(N_CORES)), trace=TRACE)
    global LAST_RESULT
    LAST_RESULT = res
    outs = [res.results[c]["out"] for c in range(N_CORES)]
    full = np.concatenate(outs, axis=0).reshape(B, H, T, T)
    return full



# revision 12
# speedup vs baseline: 1.4336x; 1.4336x over previous
"""Trainium2 Bass kernel for nn_DirectionalWedgeBias.

Computes, per (batch b, head h):
    v      = x[b].reshape(T, H, Dh)[:, h, :]          # [T, Dh]
    v_hat  = v / max(||v||_2, eps)  (row-wise)
    S      = A[h] - A[h]^T                            # [Dh, Dh]
    wedge  = (v_hat @ S) @ v_hat^T                    # [T, T]

Full shapes: x [2, 2048, 1024] f32, A [16, 64, 64] f32 -> out [2, 16, 2048, 2048] f32.

Sharding: 32 independent (b, h) pairs split 4-per-core across 8 NeuronCores
(data + head parallel; the tiny skew-symmetric S is replicated/sliced with the
heads). Host pre-slices x into per-core [4, T, Dh] blocks, forms S = A - A^T,
and re-stacks the per-core [4, T, T] results.

Per-core dataflow (Tile framework), fp16 end to end (rel err ~5e-4 vs the
2e-2 gate; f32 is kept only for the row norms):
  - x loads use a 2-rows-per-partition interleave (t = n*256 + 2*part + r) so
    each partition receives contiguous 512 B runs (full-rate descriptors
    instead of the <512 B read-modify-write class)
  - row-normalize per 512-row group (DVE square+reduce f32, ACT sqrt, DVE
    reciprocal, broadcast multiply casting to fp16) so the first transposes
    start without waiting for the whole pair
  - PE-transpose v_hat chunks into vt [64, T] fp16 (true t order, stride-2
    column scatter on the PSUM evacuation); SvT = S @ vT is scattered
    block-major so wedge lhsT slices stay contiguous
  - wedge: one fp16 matmul per [128, 512] PSUM tile in a 6-deep ring (6 of
    the 8 banks), so the ring never throttles PE on evacuation latency;
    PSUM->SBUF evacuation casts to fp16 and rotates over ACT/DVE/Pool (the
    three engines that can read PSUM), never repeating an engine twice in a
    row; each [128, 1024] half-row chunk is stored as its own fp16 DMA
  - stores are fp16 (half the f32 DMA bytes) on the three DMA queues
    SP/Pool/ACT: a chunk's store chains in order behind its own second-half
    evacuation engine, or goes to SP through a one-chunk deferral queue --
    both avoid head-of-line blocking an engine queue on a cross-engine
    dependency; the host widens fp16 back to f32
  - a greedy balancer assigns every evacuation/store by modeled ns cost
  - the final n-block evacuates and stores at 512 grain across all queues to
    collapse the pipeline drain
  - walrus encodes at most ONE semaphore wait on most instructions, so
    `_spill_waits` post-processes the Tile-scheduled BIR, hoisting excess
    waits onto preceding same-engine EventSemaphores

Cost-model (CoreSim) engine-busy: ACT/SP/Pool/DVE ~67-73 us each, PE ~63 us.
"""

import numpy as np

B = 2
T = 2048
D = 1024
H = 16
Dh = 64
N_CORES = 8
PAIRS = (B * H) // N_CORES  # 4 per core
P = 128  # SBUF partitions
NB = T // 256  # 8 n-blocks (256 rows) per pair
R = 2  # t-rows per partition within an n-block

_COMPILED = {}

# test-harness knobs (default off; harness calls kernel() with these untouched)
TRACE = False
LAST_RESULT = None

# modeled per-instruction costs (ns) for the greedy engine balancer
_EVAC_512 = {"ACT": 670.0, "DVE": 728.0, "POOL": 520.0}
_STORE_Q = 850.0  # [128, 1024] fp16 chunk = 2048 B/partition
_LOAD_PAIR = 1729.0
_NORM_GROUP = 850.0  # vsq + reduce per 512-row group ([128, 4, 64]) on DVE
_VHAT_GROUP = 460.0


def _build_nc(pairs=PAIRS, t=T, spill=True):
    _import_concourse()
    from contextlib import ExitStack

    import concourse.bass as bass
    import concourse.tile as tile
    from concourse import mybir

    f32 = mybir.dt.float32
    f16 = mybir.dt.float16
    nb = t // 256  # n-blocks per pair
    ng = t // 512  # 512-row load/norm groups per pair

    nc = bass.Bass()
    x_in = nc.declare_dram_parameter("x", [pairs, t, Dh], f32, isOutput=False)
    s_in = nc.declare_dram_parameter("s", [pairs, Dh, Dh], f32, isOutput=False)
    id_in = nc.declare_dram_parameter("ident", [P, P], f32, isOutput=False)
    out_d = nc.declare_dram_parameter("out", [pairs, t, t], f16, isOutput=True)

    busy = {"ACT": 0.0, "DVE": 0.0, "POOL": 0.0, "SP": 0.0}
    last_ev = {"e": None}

    def pick(cands, costs):
        k = min(cands, key=lambda e: busy[e] + costs[e])
        busy[k] += costs[k]
        return k

    def pick_ev(costs):
        # balanced evac choice that never repeats the previous engine, so
        # consecutive wedge-PSUM ring slots drain through different queues
        cands = [e for e in ("ACT", "DVE", "POOL") if e != last_ev["e"]]
        k = min(cands, key=lambda e: busy[e] + costs[e])
        busy[k] += costs[k]
        last_ev["e"] = k
        return k

    def charge(eng_name, cost):
        busy[eng_name] += cost

    with ExitStack() as ctx:
        tc = ctx.enter_context(tile.TileContext(nc))
        eng = {"ACT": nc.scalar, "DVE": nc.vector, "POOL": nc.gpsimd, "SP": nc.sync}

        def copy_on(e, out, in_):
            if e == "ACT":
                eng[e].copy(out, in_)
            else:
                eng[e].tensor_copy(out, in_)

        const_pool = ctx.enter_context(tc.tile_pool(name="const", bufs=1))
        stage_pool = ctx.enter_context(tc.tile_pool(name="stage", bufs=2))
        pair_pool = ctx.enter_context(tc.tile_pool(name="pair", bufs=2))
        norm_pool = ctx.enter_context(tc.tile_pool(name="norm", bufs=2))
        psw_pool = ctx.enter_context(tc.tile_pool(name="psw", bufs=6, space="PSUM"))
        pvt_pool = ctx.enter_context(tc.tile_pool(name="pvt", bufs=1, space="PSUM"))
        psv_pool = ctx.enter_context(tc.tile_pool(name="psv", bufs=1, space="PSUM"))
        out_pool = ctx.enter_context(tc.tile_pool(name="outb", bufs=6))

        # identity: DMA-landed, staged through ACT (cast to fp16) so matmuls
        # only wait on ACT
        id_dma = const_pool.tile([P, P], f32)
        nc.sync.dma_start(out=id_dma, in_=id_in[:, :])
        identity = const_pool.tile([P, P], f16)
        nc.scalar.copy(identity, id_dma)
        charge("ACT", 300.0)
        # warmup matmuls: absorb the ACT(identity) wait and hold the PE
        # p-state ramp until the first real transposes arrive
        ps_warm = psv_pool.tile([Dh, 512], f32, tag="psv")
        for _ in range(10):
            nc.tensor.matmul(
                ps_warm[:, :P],
                lhsT=identity[:, :Dh],
                rhs=identity,
                start=True,
                stop=True,
            )

        gc = (nb * R) // ng  # (n, r) chunks per 512-row group = 4
        state = {}  # per-pair tiles
        sp_defer = []  # one-chunk SP store deferral against HOL blocking

        def prep_load_norm(p):
            """Load x[p] (512 B runs per partition), row-normalize, cast fp16."""
            s_dma = stage_pool.tile([Dh, Dh], f32, tag="sdma")
            nc.scalar.dma_start(out=s_dma, in_=s_in[p])
            charge("ACT", 500.0)
            s_sb = pair_pool.tile([Dh, Dh], f16, tag="s")
            nc.scalar.copy(s_sb[:], s_dma)
            charge("ACT", 240.0)

            v_sb = pair_pool.tile([P, nb * R, Dh], f32, tag="v")
            vsq = norm_pool.tile([P, nb * R, Dh], f32, tag="vsq")
            sumsq = norm_pool.tile([P, nb * R], f32, tag="ss")
            if p > 0:
                nc.sync.dma_start(
                    out=v_sb[:].rearrange("p (n r) d -> p n r d", r=R),
                    in_=x_in[p].rearrange("(n p r) d -> p n r d", p=P, r=R),
                )
                charge("SP", _LOAD_PAIR + 150.0)
            for g in range(ng):
                if p == 0:
                    ld = ("SP", "POOL", "ACT", "POOL")[g % 4]
                    eng[ld].dma_start(
                        out=v_sb[:, g * gc : (g + 1) * gc, :].rearrange(
                            "p (n r) d -> p n r d", r=R
                        ),
                        in_=x_in[p][g * 512 : (g + 1) * 512, :].rearrange(
                            "(n p r) d -> p n r d", p=P, r=R
                        ),
                    )
                    charge(ld, _LOAD_PAIR / ng + 150.0)
                nc.vector.tensor_mul(
                    vsq[:, g * gc : (g + 1) * gc, :],
                    v_sb[:, g * gc : (g + 1) * gc, :],
                    v_sb[:, g * gc : (g + 1) * gc, :],
                )
                nc.vector.reduce_sum(
                    sumsq[:, g * gc : (g + 1) * gc],
                    vsq[:, g * gc : (g + 1) * gc, :],
                    axis=mybir.AxisListType.X,
                )
                charge("DVE", _NORM_GROUP)
            nrm = norm_pool.tile([P, nb * R], f32, tag="nrm")
            rinv = norm_pool.tile([P, nb * R], f32, tag="rinv")
            v_hat = pair_pool.tile([P, nb * R, Dh], f16, tag="vhat")
            for g in range(ng):
                sl = slice(g * gc, (g + 1) * gc)
                nc.scalar.activation(
                    nrm[:, sl], sumsq[:, sl], mybir.ActivationFunctionType.Sqrt
                )
                charge("ACT", 200.0)
                nc.vector.reciprocal(rinv[:, sl], nrm[:, sl])
                charge("DVE", 120.0)
                rb = rinv[:, sl].unsqueeze(-1).broadcast_to((P, gc, Dh))
                nc.vector.tensor_mul(v_hat[:, sl, :], v_sb[:, sl, :], rb)
                charge("DVE", _VHAT_GROUP)
            vt_sb = pair_pool.tile([Dh, t], f16, tag="vt")
            svt_sb = pair_pool.tile([Dh, t], f16, tag="svt")
            state[p] = {"s": s_sb, "vhat": v_hat, "vt": vt_sb, "svt": svt_sb}

        def prep_pe_group(p, g):
            """PE-transpose group g of v_hat into vt (true t order) and form
            that group's SvT slice (block-major)."""
            st = state[p]
            ps_vt = pvt_pool.tile([Dh, 512], f16, tag="pvt")
            for j in range(gc):
                nc.tensor.transpose(
                    ps_vt[:, j * P : (j + 1) * P],
                    st["vhat"][:, g * gc + j, :],
                    identity,
                )
            # ps_vt is block-ordered [(n, r) chunks, j]; chunk (n, r) holds
            # t = n*256 + 2*j + r -> scatter into true-t-order vt
            e = pick(("ACT", "DVE", "POOL"), _EVAC_512)
            copy_on(
                e,
                st["vt"][:, g * 512 : (g + 1) * 512].rearrange(
                    "d (n j r) -> d n r j", n=gc // R, j=P, r=R
                ),
                ps_vt.rearrange("d (n r j) -> d n r j", n=gc // R, r=R, j=P),
            )
            ps_sv = psv_pool.tile([Dh, 512], f32, tag="psv")
            nc.tensor.matmul(
                ps_sv,
                lhsT=st["s"][:],
                rhs=st["vt"][:, g * 512 : (g + 1) * 512],
                start=True,
                stop=True,
            )
            # ps_sv is true-t-ordered; scatter block-major so wedge lhsT
            # slices are contiguous: svt col (n*R + r)*128 + j <- t
            e = pick(("ACT", "DVE", "POOL"), _EVAC_512)
            copy_on(
                e,
                st["svt"][:, g * 512 : (g + 1) * 512].rearrange(
                    "d (n r j) -> d n j r", n=gc // R, r=R, j=P
                ),
                ps_sv.rearrange("d (n j r) -> d n j r", n=gc // R, j=P, r=R),
            )

        def wedge_block(p, n):
            """One 256-row n-block: 8 [128, 512] PSUM ring tiles; each is
            evacuated (fp16 cast) on a rotating engine; every [128, 1024]
            chunk is stored as its own quarter DMA."""
            st = state[p]
            ob = out_pool.tile([P, R, t], f16, tag="ob")
            dst = out_d[p][n * 256 : (n + 1) * 256, :].rearrange(
                "(j r) c -> j r c", r=R
            )
            store = {k: _STORE_Q for k in ("SP", "POOL", "ACT")}
            fine = p == pairs - 1 and n == nb - 1
            for r in range(R):
                blk = n * R + r
                for h in range(t // 1024):
                    evs = []
                    for u in range(2):
                        ps_w = psw_pool.tile([P, 512], f32, tag="psw")
                        nc.tensor.matmul(
                            ps_w,
                            lhsT=st["svt"][:, blk * P : (blk + 1) * P],
                            rhs=st["vt"][
                                :, (h * 2 + u) * 512 : (h * 2 + u + 1) * 512
                            ],
                            start=True,
                            stop=True,
                        )
                        lo = h * 1024 + u * 512
                        ev = pick_ev(_EVAC_512)
                        evs.append(ev)
                        copy_on(ev, ob[:, r, lo : lo + 512], ps_w)
                        if fine:
                            # pipeline drain: store the tail at 512 grain
                            cands = ("SP", ev) if ev != "DVE" else ("SP",)
                            e = pick(cands, {k: 500.0 for k in cands})
                            eng[e].dma_start(
                                out=dst[:, r, lo : lo + 512],
                                in_=ob[:, r, lo : lo + 512],
                            )
                    if fine:
                        continue
                    # store chains behind the second evac's engine (its first
                    # evac is already ahead in a parallel queue) or goes to SP
                    # via a one-chunk deferral -- never a third engine, which
                    # would head-of-line block on a cross-engine dependency
                    ev = evs[-1]
                    cands = ("SP", ev) if ev != "DVE" else ("SP",)
                    e = pick(cands, store)
                    if e == "SP":
                        sp_defer.append(
                            (
                                dst[:, r, h * 1024 : (h + 1) * 1024],
                                ob[:, r, h * 1024 : (h + 1) * 1024],
                            )
                        )
                        if len(sp_defer) > 1:
                            o_, i_ = sp_defer.pop(0)
                            nc.sync.dma_start(out=o_, in_=i_)
                    else:
                        eng[e].dma_start(
                            out=dst[:, r, h * 1024 : (h + 1) * 1024],
                            in_=ob[:, r, h * 1024 : (h + 1) * 1024],
                        )

        # software pipeline: pair p's wedge overlaps pair p+1's load/norm
        # (emitted first so DVE runs it early) and its transpose/Sv groups
        # (interleaved mid-wedge so the evacs drain before the wedge tail)
        prep_load_norm(0)
        for g in range(ng):
            prep_pe_group(0, g)
        for p in range(pairs):
            if p + 1 < pairs:
                prep_load_norm(p + 1)
            for n in range(nb):
                if p == pairs - 1 and n == nb - 1:
                    for o_, i_ in sp_defer:
                        nc.sync.dma_start(out=o_, in_=i_)
                    sp_defer.clear()
                wedge_block(p, n)
                if p + 1 < pairs and n - 3 in range(ng):
                    prep_pe_group(p + 1, n - 3)
            state.pop(p)
        for o_, i_ in sp_defer:
            nc.sync.dma_start(out=o_, in_=i_)

    if spill:
        _spill_waits(nc)
    nc._balancer_busy = dict(busy)
    return nc


def _spill_waits(nc, multi_ok=("EventSemaphore",), max_keep=1):
    """Walrus encodes at most one sync-wait on Matmult (embedded weight load)
    and DMACopy; move extra waits onto a preceding same-engine EventSemaphore
    (which supports many waits). The engine sequencer processes instructions
    in order, so a preceding wait is semantically identical."""
    from concourse import mybir

    n_spilled = 0
    for f in nc.m.functions:
        for bb in f.blocks:
            il = bb.instructions
            out = []
            for inst in il:
                si = getattr(inst, "sync_info", None)
                waits = list((si.on_wait if si else None) or [])
                cap = 2 if inst.opcode in multi_ok else max_keep
                if len(waits) > cap:
                    moved, keep = waits[:-max_keep], waits[-max_keep:]
                    for k in range(0, len(moved), 2):
                        es = mybir.InstEventSemaphore(
                            name=f"{inst.name}-wspill{k}",
                            engine=inst.engine,
                            ins=[],
                            outs=[],
                            sync_info=mybir.SyncInfo(
                                on_wait=moved[k : k + 2], on_update=[]
                            ),
                        )
                        out.append(es)
                    inst.sync_info = mybir.SyncInfo(
                        on_wait=keep, on_update=list(si.on_update or [])
                    )
                    n_spilled += 1
                out.append(inst)
            il[:] = out
    return n_spilled


def _import_concourse():
    try:
        import concourse  # noqa: F401
    except ImportError:
        import sys

        for p in ("/opt/trn_rl_repo", "/root/.axon_site/_ro/trn_rl_repo"):
            if p not in sys.path:
                sys.path.insert(0, p)


def _ensure_device_backend():
    """If the process pinned JAX_PLATFORMS to cpu, lift the pin so the
    NeuronCores (axon platform) are reachable for the kernel run."""
    import os

    plats = os.environ.get("JAX_PLATFORMS", "")
    if plats and "axon" not in plats and "neuron" not in plats:
        os.environ["JAX_PLATFORMS"] = ""
        try:
            import jax

            jax.extend.backend.clear_backends()
        except Exception:
            pass


def kernel(x, A, window_size=None):
    _import_concourse()
    _ensure_device_backend()
    from concourse.bass_utils import run_bass_kernel_spmd

    x = np.ascontiguousarray(x, dtype=np.float32)
    A = np.ascontiguousarray(A, dtype=np.float32)
    assert x.shape == (B, T, D) and A.shape == (H, Dh, Dh)

    nc = _COMPILED.get("nc")
    if nc is None:
        nc = _build_nc()
        _COMPILED["nc"] = nc

    # x[b, t, h*64:(h+1)*64] per (b,h) pair; pair index bh = b*H + h.
    xv = x.reshape(B, T, H, Dh).transpose(0, 2, 1, 3).reshape(B * H, T, Dh)
    S = (A - np.swapaxes(A, -1, -2)).astype(np.float32)  # replicated with heads
    S_all = np.tile(S, (B, 1, 1))
    ident = np.eye(P, dtype=np.float32)
    in_maps = []
    for c in range(N_CORES):
        sl = slice(c * PAIRS, (c + 1) * PAIRS)
        in_maps.append(
            {
                "x": np.ascontiguousarray(xv[sl]),
                "s": np.ascontiguousarray(S_all[sl]),
                "ident": ident,
            }
        )
    res = run_bass_kernel_spmd(nc, in_maps, list(range(N_CORES)), trace=TRACE)
    global LAST_RESULT
    LAST_RESULT = res
    outs = [np.asarray(res.results[c]["out"]) for c in range(N_CORES)]
    full = np.concatenate(outs, axis=0).reshape(B, H, T, T).astype(np.float32)
    return full
